# revision 18
# baseline (speedup 1.0000x reference)
# Trainium2 Bass kernel for nn_MultiHeadAttention_87024627352037.
#
# Full module: y = LayerNorm(x_q + (softmax(mask(QK^T/sqrt(nd))) V) Wo^T + bo)
# with Q/K/V projections of x_q/x_k/x_v. Shapes: B=2, S=2048, D=1024, H=16.
#
# Sharding (8 cores): core c = (batch b=c//4, head-quad g=c%4).
# Each core projects Q/K/V for its 4 heads (dv=256) over its batch,
# runs causal attention in a fully transposed layout (scoresT = K_T^T Q_T,
# no max-subtraction -- scores are O(1) -- denominator via a ones-column in
# the PV matmul), then a per-batch 4-core AllToAll re-shards ctx from
# head-sharding to row-sharding, and each core computes the output
# projection + residual + LayerNorm for its 512 rows.
#
# Perf notes vs the v1 baseline (425us):
# - softmax denominators use nc.vector.reciprocal (DVE) instead of
#   exp(-ln(d)) on ACT: the Exp<->Ln activation-table flip-flop cost 64
#   ACT_TABLE_LOADs (82us) and the 1.3us stalls kept the PE clock cold.
# - all DRAM operands are host-pre-permuted so every DMA line is >=1KB
#   contiguous (was 40k 256-512B packets, 83% DMA busy).
# - exp runs as [128,1024] activations spanning two PSUM banks.
# - the AllToAll runs in two groups of 4 (per batch), halving payload, and
#   the gather on the receive side is a plain DMA (no select/DVE pass).
# - output projection for the first half of rows is interleaved into the
#   attention tail; LayerNorm stats use fused tensor_tensor_reduce and
#   ACT Square(accum_out); the Sqrt table load happens once, after the
#   last Exp.
import sys
import types

import numpy as np

B, S, D, H = 2, 2048, 1024, 16
ND = D // H          # 64
NC = 8               # cores
HPC = H // 4         # 4 heads per core
DVC = HPC * ND       # 256 dv per core
QT = 256             # q tile (rhs free dim)
KB = 128             # k block (psum partition dim)
EPS = 1e-5
SCALE = 1.0 / np.sqrt(ND)

_cache = {}


def _install_ntff_shim():
    # antenv.axon_hooks is absent in this image; register the NTFF profile
    # hook so trace=True can capture HW exec time (harmless if unused).
    if "antenv.axon_hooks" in sys.modules:
        return
    mod = types.ModuleType("antenv.axon_hooks")
    mod._hook = None
    mod.set_axon_ntff_profile_hook = lambda h: setattr(mod, "_hook", h)
    mod.get_axon_ntff_profile_hook = lambda: mod._hook
    sys.modules["antenv.axon_hooks"] = mod
    try:
        import antenv

        antenv.axon_hooks = mod
        from trn_agent_boot.trn_boot import _ntff_profile_via_ctypes

        mod._hook = _ntff_profile_via_ctypes("/opt/axon/libaxon_pjrt.so")
    except Exception:
        pass


def _build():
    import concourse.bass as bass
    import concourse.mybir as mybir
    import concourse.tile as tile
    from concourse import bacc

    F32 = mybir.dt.float32
    F32R = mybir.dt.float32r
    BF16 = mybir.dt.bfloat16
    ADD = mybir.AluOpType.add
    MUL = mybir.AluOpType.mult
    SUB = mybir.AluOpType.subtract
    AF = mybir.ActivationFunctionType

    nc = bacc.Bacc("TRN2", target_bir_lowering=False, debug=False, num_devices=NC)

    def din(name, shape, dt=BF16):
        return nc.dram_tensor(name, shape, dt, kind="ExternalInput").ap()

    # host-pre-permuted layouts; every DMA line is contiguous in DRAM
    xtk = din("xtk", [4, 128, 8, 512])        # [r, p, cc, s] : x^T chunked
    xtv = din("xtv", [4, 128, 8, 512])
    xtq = din("xtq", [4, 128, 8, 512])
    wqT = din("wqT", [128, 8, DVC])
    wkT = din("wkT", [128, 8, DVC])
    wvT = din("wvT", [128, 8, DVC])
    woT = din("woT", [128, 8, D])
    smallc = din("smallc", [128, 288], F32)   # bq2|bk2|eps|pad|bvz(260)
    gam_bc = din("gam_bc", [128, D])          # bf16 broadcast
    bet_bc = din("bet_bc", [128, D])
    resid = din("resid", [128, 4, D], F32)    # (x_q rows + bo), p-major
    mo_in = din("mo", [128, 2 * QT])          # maskA|maskB (bf16)
    ones_r = din("ones_r", [128, 64], F32R)
    out_d = nc.dram_tensor("out", [512, D], F32, kind="ExternalOutput").ap()

    groups = [list(range(NC))]

    with nc.allow_low_precision(reason="f32r matmul operand chain"), tile.TileContext(
        nc
    ) as tc:
        with (
            tc.tile_pool(name="const", bufs=1) as cpool,
            tc.tile_pool(name="res", bufs=1) as rpool,
            tc.tile_pool(name="pt", bufs=3) as ptpool,
            tc.tile_pool(name="dn", bufs=2) as dnpool,
            tc.tile_pool(name="p3", bufs=1) as p3pool,
            tc.tile_pool(name="ln", bufs=2) as lnpool,
            tc.tile_pool(name="ps", bufs=2, space="PSUM") as PS,
            tc.tile_pool(name="dram", bufs=1, space="DRAM") as dram,
        ):
            # ---- constants / weights into SBUF ----
            smallc_sb = cpool.tile([128, 288], F32)
            ones_sb = cpool.tile([128, 64], F32R)
            mo_sb = cpool.tile([128, 2 * QT], BF16)
            wo_sb = cpool.tile([128, 8, D], BF16)
            gam_sb = cpool.tile([128, D], BF16)
            bet_sb = cpool.tile([128, D], BF16)
            bq_sb = smallc_sb[:, 0:2]
            bk_sb = smallc_sb[:, 2:4]
            eps_sb = smallc_sb[:, 4:5]
            sel_sb = smallc_sb[:, 8:16]
            bvz_sb = smallc_sb[:, 16 : 16 + HPC * (ND + 1)]

            # ---- resident activation tensors ----
            QT_sb = rpool.tile([128, 2, S], BF16)   # q^T: [dd(2x128), q]
            KT_sb = rpool.tile([128, 2, S], BF16)   # k^T: [dd(2x128), kpos]
            V_sb = rpool.tile([128, S // 128, HPC * (ND + 1)], BF16)
            ctx_sb = rpool.tile([128, 2, S], BF16)  # ctx^T: [dv(2x128), q]

            # ---- phase 1: projections ----
            p1_cm = tc.tile_pool(name="p1w", bufs=1)
            xt_cm = tc.tile_pool(name="xt", bufs=2)
            p1 = p1_cm.__enter__()
            xtpool = xt_cm.__enter__()
            wk_sb = p1.tile([128, 8, DVC], BF16)
            wv_sb = p1.tile([128, 8, DVC], BF16)
            wq_sb = p1.tile([128, 8, DVC], BF16)
            xk_sb = xtpool.tile([128, 8, S], BF16, tag="slab")
            xv_sb = xtpool.tile([128, 8, S], BF16, tag="slab")
            nc.sync.dma_start(wk_sb[:], wkT)
            nc.sync.dma_start(xk_sb[:, :, 0:512], xtk[0])
            nc.sync.dma_start(smallc_sb[:], smallc)
            for r in range(1, 4):
                nc.sync.dma_start(xk_sb[:, :, 512 * r : 512 * r + 512], xtk[r])
            nc.sync.dma_start(ones_sb[:], ones_r)
            nc.sync.dma_start(mo_sb[:], mo_in)
            nc.sync.dma_start(wv_sb[:], wvT)
            for r in range(4):
                nc.sync.dma_start(xv_sb[:, :, 512 * r : 512 * r + 512], xtv[r])
            nc.sync.dma_start(wq_sb[:], wqT)

            # K^T then Q^T (transposed projections; contraction over D in 8 chunks)
            def proj_t(w_sb, x_sb, b_sb, o_sb):
                for r in range(4):
                    for m in range(2):
                        ps = PS.tile([128, 1024], F32, tag="big")
                        for cc in range(8):
                            nc.tensor.matmul(
                                ps[:, 0:512],
                                lhsT=w_sb[:, cc, 128 * m : 128 * m + 128],
                                rhs=x_sb[:, cc, 512 * r : 512 * r + 512],
                                start=(cc == 0),
                                stop=(cc == 7),
                            )
                        nc.scalar.activation(
                            o_sb[:, m, 512 * r : 512 * r + 512],
                            ps[:, 0:512],
                            AF.Identity,
                            bias=b_sb[:, m : m + 1],
                        )

            proj_t(wk_sb, xk_sb, bk_sb, KT_sb)

            # V (natural layout [kpos, dv], per-head slots with a ones column)
            for rc4 in range(4):
                ps = PS.tile([128, 1024], F32, tag="big")
                for sub in range(4):
                    rc = 4 * rc4 + sub
                    for cc in range(8):
                        nc.tensor.matmul(
                            ps[:, 256 * sub : 256 * sub + 256],
                            lhsT=xv_sb[:, cc, 128 * rc : 128 * rc + 128],
                            rhs=wv_sb[:, cc, :],
                            start=(cc == 0),
                            stop=(cc == 7),
                        )
                for sub in range(4):
                    rc = 4 * rc4 + sub
                    nc.vector.tensor_copy(V_sb[:, rc, :], bvz_sb)
                    v_slot = V_sb[:, rc, :].rearrange("p (h x) -> p h x", x=ND + 1)[
                        :, :, 0:ND
                    ]
                    nc.vector.tensor_tensor(
                        out=v_slot,
                        in0=ps[:, 256 * sub : 256 * sub + 256].rearrange(
                            "p (h x) -> p h x", x=ND
                        ),
                        in1=v_slot,
                        op=ADD,
                    )

            # xq reuses xk's slab slot; DMA after xk's last read (K proj done)
            xq_sb = xtpool.tile([128, 8, S], BF16, tag="slab")
            for r in range(4):
                nc.sync.dma_start(xq_sb[:, :, 512 * r : 512 * r + 512], xtq[r])

            proj_t(wq_sb, xq_sb, bq_sb, QT_sb)

            xt_cm.__exit__(None, None, None)
            p1_cm.__exit__(None, None, None)

            # phase-3 constants (loaded behind the x tensors)
            nc.sync.dma_start(wo_sb[:], woT)
            nc.sync.dma_start(gam_sb[:], gam_bc)
            nc.sync.dma_start(bet_sb[:], bet_bc)

            # ---- A2A buffers (global 8-core mesh; slots for both batches) ----
            a2a_in = [
                dram.tile([NC, DVC, QT], BF16, name=f"a2a_in{i}") for i in range(2)
            ]
            a2a_out = [
                dram.tile([NC, DVC, QT], BF16, name=f"a2a_out{i}") for i in range(2)
            ]

            # phase-3 persistent tiles
            gath = [p3pool.tile([128, 8, QT], BF16, name=f"gath{i}") for i in range(2)]
            y4 = p3pool.tile([128, 4, D], F32)
            st = p3pool.tile([128, 16], F32)  # per-chunk stats columns

            def phase3_pre(ha):
                # gather: a2a_out[ha][j] / [j+4] hold ctx^T dv-chunk j for q
                # rows of tile 2g+ha from batch-0 / batch-1 sources; this
                # core's batch is selected by the sel columns (input data).
                ga = lnpool.tile([128, 8, QT], BF16, tag="ga")
                gb = lnpool.tile([128, 8, QT], BF16, tag="ga")
                nc.sync.dma_start(
                    ga[:],
                    a2a_out[ha][0:4].rearrange("j (m p) q -> p (j m) q", p=128),
                )
                nc.sync.dma_start(
                    gb[:],
                    a2a_out[ha][4:8].rearrange("j (m p) q -> p (j m) q", p=128),
                )
                for gp in range(4):
                    for m in range(2):
                        d2 = 2 * gp + m
                        t1 = lnpool.tile([128, QT], BF16, tag="t1")
                        nc.vector.tensor_scalar(
                            out=t1[:],
                            in0=ga[:, d2, :],
                            scalar1=sel_sb[:, gp : gp + 1],
                            scalar2=None,
                            op0=MUL,
                        )
                        nc.vector.scalar_tensor_tensor(
                            out=gath[ha][:, d2, :],
                            in0=gb[:, d2, :],
                            scalar=sel_sb[:, 4 + gp : 5 + gp],
                            in1=t1[:],
                            op0=MUL,
                            op1=ADD,
                        )
                for rc in range(2):
                    R = 2 * ha + rc
                    res_sb = lnpool.tile([128, D], F32, tag="res")
                    nc.sync.dma_start(res_sb[:], resid[:, R, :])
                    ps = PS.tile([128, 1024], F32, tag="big")
                    for half in range(2):
                        for d2 in range(8):
                            nc.tensor.matmul(
                                ps[:, 512 * half : 512 * half + 512],
                                lhsT=gath[ha][:, d2, 128 * rc : 128 * rc + 128],
                                rhs=wo_sb[:, d2, 512 * half : 512 * half + 512],
                                start=(d2 == 0),
                                stop=(d2 == 7),
                            )
                    # y = ps + (resid + bo);  st0 = rowsum(y)
                    # (tensor_tensor_reduce crashes HW -- probe 3; keep split)
                    nc.vector.tensor_tensor(
                        out=y4[:, R, :], in0=ps[:], in1=res_sb[:], op=ADD
                    )
                    nc.vector.reduce_sum(
                        st[:, 4 * R : 4 * R + 1],
                        y4[:, R, :],
                        axis=mybir.AxisListType.X,
                    )
                    # st1 = rowsum(y^2)  (Square lives in every ACT table set)
                    sq = lnpool.tile([128, 1024], BF16, tag="sq")
                    nc.scalar.activation(
                        sq[:],
                        y4[:, R, :],
                        AF.Square,
                        accum_out=st[:, 4 * R + 1 : 4 * R + 2],
                    )

            def phase3_fin(R):
                c0 = st[:, 4 * R : 4 * R + 1]       # sum y
                c1 = st[:, 4 * R + 1 : 4 * R + 2]   # sum y^2
                nmu = st[:, 4 * R + 2 : 4 * R + 3]  # -mean
                rsd = st[:, 4 * R + 3 : 4 * R + 4]  # 1/std
                nc.vector.tensor_scalar_mul(nmu, c0, -1.0 / D)
                # var = E[y^2] - mu^2  (in two tiny steps, sq then sub)
                v0 = stv[:, 2 * R : 2 * R + 1]
                v1 = stv[:, 2 * R + 1 : 2 * R + 2]
                nc.vector.tensor_scalar_mul(v0, c1, 1.0 / D)
                nc.vector.tensor_tensor(out=v1, in0=nmu, in1=nmu, op=MUL)
                nc.vector.tensor_tensor(out=v0, in0=v0, in1=v1, op=SUB)
                nc.scalar.activation(v1, v0, AF.Sqrt, bias=eps_sb)
                nc.vector.reciprocal(out=rsd, in_=v1)
                nc.vector.tensor_tensor(out=nmu, in0=nmu, in1=rsd, op=MUL)
                # yn = (y - mu) / std  via one ACT; then gamma/beta on DVE
                yn = lnpool.tile([128, 1024], BF16, tag="yn")
                nc.scalar.activation(
                    yn[:], y4[:, R, :], AF.Identity, bias=nmu, scale=rsd
                )
                yg = lnpool.tile([128, 1024], BF16, tag="yg")
                nc.vector.tensor_tensor(out=yg[:], in0=yn[:], in1=gam_sb[:], op=MUL)
                # final beta-add overwrites the y4 chunk in place (f32 out)
                nc.vector.tensor_tensor(
                    out=y4[:, R, :], in0=yg[:], in1=bet_sb[:], op=ADD
                )
                nc.sync.dma_start(out_d[128 * R : 128 * R + 128, :], y4[:, R, :])

            stv = p3pool.tile([128, 8], F32)

            # ---- phase 2: attention (even q-tiles first, then odd) ----
            # Software-pipelined: for each tile the (scores -> exp -> PV)
            # units issue the NEXT unit's score matmuls before the pending
            # unit's PV matmuls, so the PE never head-of-line blocks on the
            # exp.  Each head's softmax finalize (reciprocal on DVE,
            # ones-broadcast matmul, ctx scale) issues as soon as that
            # head's PV accumulation stops, overlapping later heads.
            for t in (0, 2, 4, 6, 1, 3, 5, 7):
                nblk_t = 2 * (t + 1)   # 128-k blocks for this q tile
                nj2 = (nblk_t + 3) // 4
                cps_list = [
                    PS.tile([128, QT], F32, tag="cps", bufs=4, name=f"cps{t}_{h}")
                    for h in range(HPC)
                ]
                rcb = dnpool.tile([128, HPC * QT], F32R, tag="rcb")
                bcs = dnpool.tile([64, HPC * QT], F32, tag="bcs")

                def issue_s(h, j2):
                    po = 64 * (h % 2)
                    hc = h // 2
                    kb0 = 4 * j2
                    nblk = min(4, nblk_t - kb0)
                    w = 256 * nblk
                    q_rhs = QT_sb[po : po + 64, hc, QT * t : QT * t + QT]
                    sps = PS.tile([128, 1024], F32, tag="big", name=f"sps{t}_{h}_{j2}")
                    for u in range(nblk):
                        j = kb0 + u
                        nc.tensor.matmul(
                            sps[:, 256 * u : 256 * u + 256],
                            lhsT=KT_sb[po : po + 64, hc, 128 * j : 128 * j + 128],
                            rhs=q_rhs,
                            start=True,
                            stop=True,
                        )
                    pt = ptpool.tile([128, 1024], BF16, tag="pt", name=f"pt{t}_{h}_{j2}")
                    nc.scalar.activation(pt[:, 0:w], sps[:, 0:w], AF.Exp, scale=SCALE)
                    if j2 == nj2 - 1:
                        # causal mask on the two diagonal k-blocks (GpSimd --
                        # keeps the DVE free for the softmax denominators)
                        nc.gpsimd.tensor_tensor(
                            out=pt[:, w - 512 : w],
                            in0=pt[:, w - 512 : w],
                            in1=mo_sb,
                            op=MUL,
                        )
                    return (h, j2, pt)

                def issue_pv(unit):
                    h, j2, pt = unit
                    kb0 = 4 * j2
                    nblk = min(4, nblk_t - kb0)
                    for u in range(nblk):
                        j = kb0 + u
                        nc.tensor.matmul(
                            cps_list[h][0 : ND + 1, :],
                            lhsT=V_sb[:, j, (ND + 1) * h : (ND + 1) * (h + 1)],
                            rhs=pt[:, 256 * u : 256 * u + 256],
                            start=(j == 0),
                            stop=(j == nblk_t - 1),
                        )

                def head_fin(h):
                    po = 64 * (h % 2)
                    hc = h // 2
                    nc.vector.reciprocal(
                        out=rcb[64:65, QT * h : QT * h + QT],
                        in_=cps_list[h][64:65, :],
                    )
                    bps = PS.tile([64, QT], F32, tag="big", name=f"bps{t}_{h}")
                    nc.tensor.matmul(
                        bps[:],
                        lhsT=ones_sb[64:65, 0:64],
                        rhs=rcb[64:65, QT * h : QT * h + QT],
                        start=True,
                        stop=True,
                    )
                    nc.vector.tensor_copy(bcs[:, QT * h : QT * h + QT], bps[:])
                    nc.vector.tensor_tensor(
                        out=ctx_sb[po : po + 64, hc, QT * t : QT * t + QT],
                        in0=cps_list[h][0:64, :],
                        in1=bcs[:, QT * h : QT * h + QT],
                        op=MUL,
                    )

                pend = None
                for h in range(HPC):
                    for j2 in range(nj2):
                        unit = issue_s(h, j2)
                        if pend is not None:
                            issue_pv(pend)
                            if pend[1] == nj2 - 1:
                                head_fin(pend[0])
                        pend = unit
                issue_pv(pend)
                head_fin(pend[0])

                # ship this q-tile's ctx block (dup-write to both batch slots)
                ha, gp = t % 2, t // 2
                for m in range(2):
                    src = ctx_sb[:, m, QT * t : QT * t + QT]
                    nc.sync.dma_start(a2a_in[ha][gp, 128 * m : 128 * m + 128, :], src)
                    nc.sync.dma_start(
                        a2a_in[ha][gp + 4, 128 * m : 128 * m + 128, :], src
                    )
                if t == 6:
                    nc.gpsimd.collective_compute(
                        "AllToAll",
                        mybir.AluOpType.bypass,
                        replica_groups=groups,
                        ins=[a2a_in[0].opt()],
                        outs=[a2a_out[0].opt()],
                    )
                elif t == 7:
                    nc.gpsimd.collective_compute(
                        "AllToAll",
                        mybir.AluOpType.bypass,
                        replica_groups=groups,
                        ins=[a2a_in[1].opt()],
                        outs=[a2a_out[1].opt()],
                    )

            # ---- phase 3 tail: out-proj + LayerNorm (Sqrt after last Exp) ----
            phase3_pre(0)
            phase3_fin(0)
            phase3_fin(1)
            phase3_pre(1)
            phase3_fin(2)
            phase3_fin(3)

    nc.compile()
    return nc


def _prep_inputs(x_q, x_k, x_v, mask, Wq, bq, Wk, bk, Wv, bv, Wo, bo, gamma, beta):
    import ml_dtypes

    f = np.float32
    bf = ml_dtypes.bfloat16
    maskA = np.zeros((KB, QT), f)
    maskB = np.zeros((KB, QT), f)
    for i in range(KB):
        maskA[i, i:] = 1.0
        if i + 128 < QT:
            maskB[i, i + 128:] = 1.0
    mo = np.concatenate([maskA, maskB], axis=1).astype(bf)

    def chunk_xt(x_b):
        # x_b: [S, D] -> x^T [D, S] -> [r=4][p=128][cc=8][512]
        xt = np.ascontiguousarray(x_b.T.astype(bf))
        return np.ascontiguousarray(
            xt.reshape(8, 128, 4, 512).transpose(2, 1, 0, 3)
        )

    def perm_w(Wslice, ncols):
        # [ncols, D] weight slice -> W^T [D, ncols] -> [p=128][cc=8][ncols]
        wt = np.ascontiguousarray(Wslice.T.astype(bf))
        return np.ascontiguousarray(wt.reshape(8, 128, ncols).transpose(1, 0, 2))

    in_maps = []
    for c in range(NC):
        b, g = c // 4, c % 4
        dv = slice(DVC * g, DVC * (g + 1))
        smallc = np.zeros((128, 288), f)
        smallc[:, 0:2] = bq[dv].astype(f).reshape(2, 128).T
        smallc[:, 2:4] = bk[dv].astype(f).reshape(2, 128).T
        smallc[:, 4] = EPS
        smallc[:, 8 + 4 * b : 12 + 4 * b] = 1.0  # batch-select columns 8..15
        bvz = np.zeros((HPC, ND + 1), f)
        bvz[:, 0:ND] = bv[dv].astype(f).reshape(HPC, ND)
        bvz[:, ND] = 1.0
        smallc[:, 16 : 16 + HPC * (ND + 1)] = np.broadcast_to(
            bvz.reshape(-1), (128, HPC * (ND + 1))
        )
        res = (
            x_q[b, 512 * g : 512 * (g + 1), :].astype(f) + bo.astype(f)[None, :]
        )  # [512, D]
        res = np.ascontiguousarray(res.reshape(4, 128, D).transpose(1, 0, 2))
        in_maps.append(
            {
                "xtq": chunk_xt(x_q[b]),
                "xtk": chunk_xt(x_k[b]),
                "xtv": chunk_xt(x_v[b]),
                "wqT": perm_w(Wq[dv, :], DVC),
                "wkT": perm_w(Wk[dv, :], DVC),
                "wvT": perm_w(Wv[dv, :], DVC),
                "woT": perm_w(Wo, D),
                "smallc": smallc,
                "gam_bc": np.broadcast_to(gamma.astype(bf), (128, D)).copy(),
                "bet_bc": np.broadcast_to(beta.astype(bf), (128, D)).copy(),
                "resid": res,
                "mo": mo,
                "ones_r": np.ones((128, 64), f),
            }
        )
    return in_maps


def kernel(x_q, x_k, x_v, mask, Wq, bq, Wk, bk, Wv, bv, Wo, bo, gamma, beta):
    _install_ntff_shim()
    from concourse.bass_utils import run_bass_kernel_spmd

    x_q, x_k, x_v = np.asarray(x_q), np.asarray(x_k), np.asarray(x_v)
    mask = np.asarray(mask)
    # this kernel implements causal attention structurally; verify the mask
    causal = np.tril(np.ones((S, S), mask.dtype))
    assert np.array_equal(mask.reshape(S, S), causal), "kernel specialized for causal mask"

    if "nc" not in _cache:
        _cache["nc"] = _build()
    nc = _cache["nc"]

    in_maps = _prep_inputs(
        x_q, x_k, x_v, mask,
        np.asarray(Wq), np.asarray(bq), np.asarray(Wk), np.asarray(bk),
        np.asarray(Wv), np.asarray(bv), np.asarray(Wo), np.asarray(bo),
        np.asarray(gamma), np.asarray(beta),
    )
    res = run_bass_kernel_spmd(nc, in_maps, list(range(NC)))
    _cache["last_results"] = res

    out = np.empty((B, S, D), np.float32)
    for c in range(NC):
        b, g = c // 4, c % 4
        out[b, 512 * g : 512 * (g + 1), :] = res.results[c]["out"]
    return out


# revision 32
# speedup vs baseline: 1.3233x; 1.3233x over previous
# Trainium2 Bass kernel for nn_MultiHeadAttention_87024627352037.
#
# Full module: y = LayerNorm(x_q + (softmax(mask(QK^T/sqrt(nd))) V) Wo^T + bo)
# with Q/K/V projections of x_q/x_k/x_v. Shapes: B=2, S=2048, D=1024, H=16.
#
# Sharding (8 cores): core c = (batch b=c//4, head-quad g=c%4).
# Each core projects Q/K/V for its 4 heads (dv=256) over its batch,
# runs causal attention in a fully transposed layout (scoresT = K_T^T Q_T,
# no max-subtraction -- scores are O(1) -- denominator via a ones-column in
# the PV matmul), then a per-batch 4-core AllToAll re-shards ctx from
# head-sharding to row-sharding, and each core computes the output
# projection + residual + LayerNorm for its 512 rows.
#
# Perf notes vs the v1 baseline (425us):
# - softmax denominators use nc.vector.reciprocal (DVE) instead of
#   exp(-ln(d)) on ACT: the Exp<->Ln activation-table flip-flop cost 64
#   ACT_TABLE_LOADs (82us) and the 1.3us stalls kept the PE clock cold.
# - all DRAM operands are host-pre-permuted so every DMA line is >=1KB
#   contiguous (was 40k 256-512B packets, 83% DMA busy).
# - exp runs as [128,1024] activations spanning two PSUM banks.
# - the AllToAll runs in two groups of 4 (per batch), halving payload, and
#   the gather on the receive side is a plain DMA (no select/DVE pass).
# - output projection for the first half of rows is interleaved into the
#   attention tail; LayerNorm stats use fused tensor_tensor_reduce and
#   ACT Square(accum_out); the Sqrt table load happens once, after the
#   last Exp.
import sys
import types

import numpy as np

B, S, D, H = 2, 2048, 1024, 16
ND = D // H          # 64
NC = 8               # cores
HPC = H // 4         # 4 heads per core
DVC = HPC * ND       # 256 dv per core
QT = 256             # q tile (rhs free dim)
KB = 128             # k block (psum partition dim)
EPS = 1e-5
SCALE = 1.0 / np.sqrt(ND)

_cache = {}


def _install_ntff_shim():
    # antenv.axon_hooks is absent in this image; register the NTFF profile
    # hook so trace=True can capture HW exec time (harmless if unused).
    if "antenv.axon_hooks" in sys.modules:
        return
    mod = types.ModuleType("antenv.axon_hooks")
    mod._hook = None
    mod.set_axon_ntff_profile_hook = lambda h: setattr(mod, "_hook", h)
    mod.get_axon_ntff_profile_hook = lambda: mod._hook
    sys.modules["antenv.axon_hooks"] = mod
    try:
        import antenv

        antenv.axon_hooks = mod
        from trn_agent_boot.trn_boot import _ntff_profile_via_ctypes

        mod._hook = _ntff_profile_via_ctypes("/opt/axon/libaxon_pjrt.so")
    except Exception:
        pass


def _build():
    import concourse.bass as bass
    import concourse.mybir as mybir
    import concourse.tile as tile
    from concourse import bacc

    F32 = mybir.dt.float32
    F32R = mybir.dt.float32r
    BF16 = mybir.dt.bfloat16
    ADD = mybir.AluOpType.add
    MUL = mybir.AluOpType.mult
    SUB = mybir.AluOpType.subtract
    AF = mybir.ActivationFunctionType

    nc = bacc.Bacc("TRN2", target_bir_lowering=False, debug=False, num_devices=NC)

    # Pin the activation-table choice: this kernel only uses Exp/Ln (both in
    # natural_log_exp_and_others) plus Identity/Square (in every set) and one
    # trailing Sqrt.  Left to itself the table-load pass alternates between
    # the exp-only and ln-only sets, inserting a ~1.3us ACT_TABLE_LOAD per
    # switch (64 loads / 82us in the v1 baseline).  Restricting the candidate
    # list to a set containing both makes a switch impossible.
    import types as _types

    from concourse.hw_specs import get_activation_tables as _gat
    import bass_rust as _bass_rust

    _tables = _gat(nc.m.arch)
    _keep = ("natural_log_exp_and_others", "sqrt_and_others")
    # keep list positions (set id = index into act_info.json); empty the
    # sets we don't want so the pass can never choose them
    _filtered = [
        (k, (v if k in _keep else set())) for k, v in _tables.items()
    ]

    def _insert_act_table_loads(self):
        has_activation = any(
            isinstance(i, mybir.InstActivation)
            for b in self.main_func.blocks
            for i in b.instructions
        )
        if not has_activation:
            return
        _bass_rust.insert_act_table_loads(self, _filtered)

    nc.insert_act_table_loads = _types.MethodType(_insert_act_table_loads, nc)

    def din(name, shape, dt=BF16):
        return nc.dram_tensor(name, shape, dt, kind="ExternalInput").ap()

    # host-pre-permuted layouts; every DMA line is contiguous in DRAM
    xtk = din("xtk", [4, 128, 8, 512])        # [r, p, cc, s] : x^T chunked
    xtv = din("xtv", [4, 128, 8, 512])
    xtq = din("xtq", [4, 128, 8, 512])
    wqT = din("wqT", [128, 8, DVC])
    wkT = din("wkT", [128, 8, DVC])
    wvT = din("wvT", [128, 8, DVC])
    woT = din("woT", [128, 8, D])
    smallc = din("smallc", [128, 288], F32)   # bq2|bk2|eps|pad|bvz(260)
    gam_bc = din("gam_bc", [128, D])          # bf16 broadcast
    bet_bc = din("bet_bc", [128, D])
    resid = din("resid", [128, 4, D], F32)    # (x_q rows + bo), p-major
    mo_in = din("mo", [128, 4 * QT])          # maskA|maskB|maskA|maskB (bf16)
    ones_r = din("ones_r", [128, 64], F32R)
    out_d = nc.dram_tensor("out", [512, D], F32, kind="ExternalOutput").ap()

    groups = [list(range(NC))]

    with nc.allow_low_precision(reason="f32r matmul operand chain"), tile.TileContext(
        nc
    ) as tc:
        with (
            tc.tile_pool(name="const", bufs=1) as cpool,
            tc.tile_pool(name="res", bufs=1) as rpool,
            tc.tile_pool(name="pt", bufs=3) as ptpool,
            tc.tile_pool(name="dn", bufs=2) as dnpool,
            tc.tile_pool(name="p3", bufs=1) as p3pool,
            tc.tile_pool(name="ln", bufs=2) as lnpool,
            tc.tile_pool(name="ps", bufs=2, space="PSUM") as PS,
            tc.tile_pool(name="dram", bufs=1, space="DRAM") as dram,
        ):
            # ---- constants / weights into SBUF ----
            smallc_sb = cpool.tile([128, 288], F32)
            ones_sb = cpool.tile([128, 64], F32R)
            mo_sb = cpool.tile([128, 4 * QT], BF16)
            wo_sb = cpool.tile([128, 8, D], BF16)
            gam_sb = cpool.tile([128, D], BF16)
            bet_sb = cpool.tile([128, D], BF16)
            bq_sb = smallc_sb[:, 0:2]
            bk_sb = smallc_sb[:, 2:4]
            eps_sb = smallc_sb[:, 4:5]
            sel_sb = smallc_sb[:, 8:16]
            bvz_sb = smallc_sb[:, 16 : 16 + HPC * (ND + 1)]

            # ---- resident activation tensors ----
            QT_sb = rpool.tile([128, 2, S], BF16)   # q^T: [dd(2x128), q]
            KT_sb = rpool.tile([128, 2, S], BF16)   # k^T: [dd(2x128), kpos]
            V_sb = rpool.tile([128, S // 128, HPC * (ND + 1)], BF16)
            ctx_sb = rpool.tile([128, 2, S], BF16)  # ctx^T: [dv(2x128), q]

            # ---- phase 1: projections ----
            p1_cm = tc.tile_pool(name="p1w", bufs=1)
            xt_cm = tc.tile_pool(name="xt", bufs=2)
            p1 = p1_cm.__enter__()
            xtpool = xt_cm.__enter__()
            wk_sb = p1.tile([128, 8, DVC], BF16)
            wv_sb = p1.tile([128, 8, DVC], BF16)
            wq_sb = p1.tile([128, 8, DVC], BF16)
            xk_sb = xtpool.tile([128, 8, S], BF16, tag="slab")
            xv_sb = xtpool.tile([128, 8, S], BF16, tag="slab")
            nc.sync.dma_start(wk_sb[:], wkT)
            nc.sync.dma_start(xk_sb[:, :, 0:512], xtk[0])
            nc.sync.dma_start(smallc_sb[:], smallc)
            for r in range(1, 4):
                nc.sync.dma_start(xk_sb[:, :, 512 * r : 512 * r + 512], xtk[r])
            nc.sync.dma_start(ones_sb[:], ones_r)
            nc.sync.dma_start(mo_sb[:], mo_in)
            nc.sync.dma_start(wv_sb[:], wvT)
            for r in range(4):
                nc.sync.dma_start(xv_sb[:, :, 512 * r : 512 * r + 512], xtv[r])
            nc.sync.dma_start(wq_sb[:], wqT)

            # K^T then Q^T (transposed projections; contraction over D in 8 chunks)
            def proj_t(w_sb, x_sb, b_sb, o_sb):
                for r in range(4):
                    for m in range(2):
                        ps = PS.tile([128, 1024], F32, tag="big")
                        for cc in range(8):
                            nc.tensor.matmul(
                                ps[:, 0:512],
                                lhsT=w_sb[:, cc, 128 * m : 128 * m + 128],
                                rhs=x_sb[:, cc, 512 * r : 512 * r + 512],
                                start=(cc == 0),
                                stop=(cc == 7),
                            )
                        nc.scalar.activation(
                            o_sb[:, m, 512 * r : 512 * r + 512],
                            ps[:, 0:512],
                            AF.Identity,
                            bias=b_sb[:, m : m + 1],
                        )

            proj_t(wk_sb, xk_sb, bk_sb, KT_sb)

            # V (natural layout [kpos, dv], per-head slots with a ones column)
            for rc4 in range(4):
                ps = PS.tile([128, 1024], F32, tag="big")
                for sub in range(4):
                    rc = 4 * rc4 + sub
                    for cc in range(8):
                        nc.tensor.matmul(
                            ps[:, 256 * sub : 256 * sub + 256],
                            lhsT=xv_sb[:, cc, 128 * rc : 128 * rc + 128],
                            rhs=wv_sb[:, cc, :],
                            start=(cc == 0),
                            stop=(cc == 7),
                        )
                for sub in range(4):
                    rc = 4 * rc4 + sub
                    nc.vector.tensor_copy(V_sb[:, rc, :], bvz_sb)
                    v_slot = V_sb[:, rc, :].rearrange("p (h x) -> p h x", x=ND + 1)[
                        :, :, 0:ND
                    ]
                    nc.vector.tensor_tensor(
                        out=v_slot,
                        in0=ps[:, 256 * sub : 256 * sub + 256].rearrange(
                            "p (h x) -> p h x", x=ND
                        ),
                        in1=v_slot,
                        op=ADD,
                    )

            # xq reuses xk's slab slot; DMA after xk's last read (K proj done)
            xq_sb = xtpool.tile([128, 8, S], BF16, tag="slab")
            for r in range(4):
                nc.sync.dma_start(xq_sb[:, :, 512 * r : 512 * r + 512], xtq[r])

            proj_t(wq_sb, xq_sb, bq_sb, QT_sb)

            xt_cm.__exit__(None, None, None)
            p1_cm.__exit__(None, None, None)

            # phase-3 constants (loaded behind the x tensors)
            nc.sync.dma_start(wo_sb[:], woT)
            nc.sync.dma_start(gam_sb[:], gam_bc)
            nc.sync.dma_start(bet_sb[:], bet_bc)

            # ---- A2A buffers (global 8-core mesh; slots for both batches) ----
            a2a_in = [
                dram.tile([NC, DVC, QT], BF16, name=f"a2a_in{i}") for i in range(2)
            ]
            a2a_out = [
                dram.tile([NC, DVC, QT], BF16, name=f"a2a_out{i}") for i in range(2)
            ]

            # phase-3 persistent tiles
            gath = [p3pool.tile([128, 8, QT], BF16, name=f"gath{i}") for i in range(2)]
            y4 = p3pool.tile([128, 4, D], F32)
            st = p3pool.tile([128, 16], F32)  # per-chunk stats columns

            def phase3_pre(ha):
                # gather: a2a_out[ha][j] / [j+4] hold ctx^T dv-chunk j for q
                # rows of tile 2g+ha from batch-0 / batch-1 sources; this
                # core's batch is selected by the sel columns (input data).
                ga = lnpool.tile([128, 8, QT], BF16, tag="ga")
                gb = lnpool.tile([128, 8, QT], BF16, tag="ga")
                nc.sync.dma_start(
                    ga[:],
                    a2a_out[ha][0:4].rearrange("j (m p) q -> p (j m) q", p=128),
                )
                nc.sync.dma_start(
                    gb[:],
                    a2a_out[ha][4:8].rearrange("j (m p) q -> p (j m) q", p=128),
                )
                for gp in range(4):
                    for m in range(2):
                        d2 = 2 * gp + m
                        t1 = lnpool.tile([128, QT], BF16, tag="t1")
                        nc.vector.tensor_scalar(
                            out=t1[:],
                            in0=ga[:, d2, :],
                            scalar1=sel_sb[:, gp : gp + 1],
                            scalar2=None,
                            op0=MUL,
                        )
                        nc.vector.scalar_tensor_tensor(
                            out=gath[ha][:, d2, :],
                            in0=gb[:, d2, :],
                            scalar=sel_sb[:, 4 + gp : 5 + gp],
                            in1=t1[:],
                            op0=MUL,
                            op1=ADD,
                        )
                for rc in range(2):
                    R = 2 * ha + rc
                    res_sb = lnpool.tile([128, D], F32, tag="res")
                    nc.sync.dma_start(res_sb[:], resid[:, R, :])
                    ps = PS.tile([128, 1024], F32, tag="big")
                    for half in range(2):
                        for d2 in range(8):
                            nc.tensor.matmul(
                                ps[:, 512 * half : 512 * half + 512],
                                lhsT=gath[ha][:, d2, 128 * rc : 128 * rc + 128],
                                rhs=wo_sb[:, d2, 512 * half : 512 * half + 512],
                                start=(d2 == 0),
                                stop=(d2 == 7),
                            )
                    # y = ps + (resid + bo);  st0 = rowsum(y)
                    # (tensor_tensor_reduce crashes HW -- probe 3; keep split)
                    nc.vector.tensor_tensor(
                        out=y4[:, R, :], in0=ps[:], in1=res_sb[:], op=ADD
                    )
                    nc.vector.reduce_sum(
                        st[:, 4 * R : 4 * R + 1],
                        y4[:, R, :],
                        axis=mybir.AxisListType.X,
                    )
                    # st1 = rowsum(y^2)  (Square lives in every ACT table set)
                    sq = lnpool.tile([128, 1024], BF16, tag="sq")
                    nc.scalar.activation(
                        sq[:],
                        y4[:, R, :],
                        AF.Square,
                        accum_out=st[:, 4 * R + 1 : 4 * R + 2],
                    )

            def phase3_fin(R):
                c0 = st[:, 4 * R : 4 * R + 1]       # sum y
                c1 = st[:, 4 * R + 1 : 4 * R + 2]   # sum y^2
                nmu = st[:, 4 * R + 2 : 4 * R + 3]  # -mean
                rsd = st[:, 4 * R + 3 : 4 * R + 4]  # 1/std
                nc.vector.tensor_scalar_mul(nmu, c0, -1.0 / D)
                # var = E[y^2] - mu^2  (in two tiny steps, sq then sub)
                v0 = stv[:, 2 * R : 2 * R + 1]
                v1 = stv[:, 2 * R + 1 : 2 * R + 2]
                nc.vector.tensor_scalar_mul(v0, c1, 1.0 / D)
                nc.vector.tensor_tensor(out=v1, in0=nmu, in1=nmu, op=MUL)
                nc.vector.tensor_tensor(out=v0, in0=v0, in1=v1, op=SUB)
                nc.scalar.activation(v1, v0, AF.Sqrt, bias=eps_sb)
                nc.vector.reciprocal(out=rsd, in_=v1)
                nc.vector.tensor_tensor(out=nmu, in0=nmu, in1=rsd, op=MUL)
                # yn = (y - mu) / std  via one ACT; then gamma/beta on DVE
                yn = lnpool.tile([128, 1024], BF16, tag="yn")
                nc.scalar.activation(
                    yn[:], y4[:, R, :], AF.Identity, bias=nmu, scale=rsd
                )
                yg = lnpool.tile([128, 1024], BF16, tag="yg")
                nc.vector.tensor_tensor(out=yg[:], in0=yn[:], in1=gam_sb[:], op=MUL)
                # final beta-add overwrites the y4 chunk in place (f32 out)
                nc.vector.tensor_tensor(
                    out=y4[:, R, :], in0=yg[:], in1=bet_sb[:], op=ADD
                )
                nc.sync.dma_start(out_d[128 * R : 128 * R + 128, :], y4[:, R, :])

            stv = p3pool.tile([128, 8], F32)

            # ---- phase 2: attention (even q-tiles first, then odd) ----
            # Software-pipelined: for each tile the (scores -> exp -> PV)
            # units issue the NEXT unit's score matmuls before the pending
            # unit's PV matmuls, so the PE never head-of-line blocks on the
            # exp.  Each head's softmax finalize (reciprocal on DVE,
            # ones-broadcast matmul, ctx scale) issues as soon as that
            # head's PV accumulation stops, overlapping later heads.
            for t in (0, 2, 4, 6, 1, 3, 5, 7):
                nblk_t = 2 * (t + 1)   # 128-k blocks for this q tile
                cps_list = [
                    PS.tile([128, QT], F32, tag="cps", bufs=4, name=f"cps{t}_{h}")
                    for h in range(HPC)
                ]
                bcs = dnpool.tile([64, HPC * QT], F32, tag="bcs")
                lnb = dnpool.tile([128, HPC * QT], F32, tag="lnb", bufs=1)
                rcb = dnpool.tile([128, HPC * QT], F32R, tag="rcb", bufs=1)

                def issue_s(p, jp):
                    # pair p = heads (2p at rows 0:64, 2p+1 at rows 64:128);
                    # the alternating row-group matmuls run concurrently on
                    # the PE (auto tile_position from base_partition).
                    sps = PS.tile([128, 1024], F32, tag="big", name=f"sps{t}_{p}_{jp}")
                    q0 = QT_sb[0:64, p, QT * t : QT * t + QT]
                    q1 = QT_sb[64:128, p, QT * t : QT * t + QT]
                    for u in range(2):
                        j = 2 * jp + u
                        nc.tensor.matmul(
                            sps[:, 256 * u : 256 * u + 256],
                            lhsT=KT_sb[0:64, p, 128 * j : 128 * j + 128],
                            rhs=q0,
                            start=True,
                            stop=True,
                        )
                        nc.tensor.matmul(
                            sps[:, 512 + 256 * u : 512 + 256 * u + 256],
                            lhsT=KT_sb[64:128, p, 128 * j : 128 * j + 128],
                            rhs=q1,
                            start=True,
                            stop=True,
                        )
                    pt = ptpool.tile([128, 1024], BF16, tag="pt", name=f"pt{t}_{p}_{jp}")
                    nc.scalar.activation(pt[:], sps[:], AF.Exp, scale=SCALE)
                    if jp == t:
                        # causal mask on the diagonal k-block pair, both heads
                        nc.vector.tensor_tensor(
                            out=pt[:], in0=pt[:], in1=mo_sb, op=MUL
                        )
                    return (p, jp, pt)

                def issue_pv(unit):
                    p, jp, pt = unit
                    for u in range(2):
                        j = 2 * jp + u
                        for s, h in enumerate((2 * p, 2 * p + 1)):
                            nc.tensor.matmul(
                                cps_list[h][0 : ND + 1, :],
                                lhsT=V_sb[:, j, (ND + 1) * h : (ND + 1) * (h + 1)],
                                rhs=pt[:, 512 * s + 256 * u : 512 * s + 256 * u + 256],
                                start=(j == 0),
                                stop=(j == nblk_t - 1),
                            )

                def head_fin(h):
                    # per-head Ln of the denominator row, issued as soon as
                    # this head's PV accumulation stops (spread across tile)
                    nc.scalar.activation(
                        lnb[64:65, QT * h : QT * h + QT],
                        cps_list[h][64:65, :],
                        AF.Ln,
                    )

                def tile_fin():
                    # 1/d = exp(-ln d) batched for all 4 heads, broadcast to
                    # 64 partitions via a rank-1 f32r matmul, then scale ctx
                    nc.scalar.activation(
                        rcb[64:65, :], lnb[64:65, :], AF.Exp, scale=-1.0
                    )
                    bps = PS.tile([64, HPC * QT], F32, tag="big", name=f"bps{t}")
                    for n in range(2):
                        nc.tensor.matmul(
                            bps[:, 512 * n : 512 * n + 512],
                            lhsT=ones_sb[64:65, 0:64],
                            rhs=rcb[64:65, 512 * n : 512 * n + 512],
                            start=True,
                            stop=True,
                        )
                    nc.vector.tensor_copy(bcs[:], bps[:])
                    for h in range(HPC):
                        po = 64 * (h % 2)
                        hc = h // 2
                        nc.vector.tensor_tensor(
                            out=ctx_sb[po : po + 64, hc, QT * t : QT * t + QT],
                            in0=cps_list[h][0:64, :],
                            in1=bcs[:, QT * h : QT * h + QT],
                            op=MUL,
                        )

                pend = None
                for p in range(2):
                    for jp in range(t + 1):
                        unit = issue_s(p, jp)
                        if pend is not None:
                            issue_pv(pend)
                            if pend[1] == t:
                                head_fin(2 * pend[0])
                                head_fin(2 * pend[0] + 1)
                        pend = unit
                issue_pv(pend)
                head_fin(2 * pend[0])
                head_fin(2 * pend[0] + 1)
                tile_fin()

                # ship this q-tile's ctx block (dup-write to both batch slots)
                ha, gp = t % 2, t // 2
                for m in range(2):
                    src = ctx_sb[:, m, QT * t : QT * t + QT]
                    nc.sync.dma_start(a2a_in[ha][gp, 128 * m : 128 * m + 128, :], src)
                    nc.sync.dma_start(
                        a2a_in[ha][gp + 4, 128 * m : 128 * m + 128, :], src
                    )
                if t == 6:
                    nc.gpsimd.collective_compute(
                        "AllToAll",
                        mybir.AluOpType.bypass,
                        replica_groups=groups,
                        ins=[a2a_in[0].opt()],
                        outs=[a2a_out[0].opt()],
                    )
                elif t == 7:
                    nc.gpsimd.collective_compute(
                        "AllToAll",
                        mybir.AluOpType.bypass,
                        replica_groups=groups,
                        ins=[a2a_in[1].opt()],
                        outs=[a2a_out[1].opt()],
                    )

            # ---- phase 3 tail: out-proj + LayerNorm (Sqrt after last Exp) ----
            phase3_pre(0)
            phase3_fin(0)
            phase3_fin(1)
            phase3_pre(1)
            phase3_fin(2)
            phase3_fin(3)

    nc.compile()
    return nc


def _prep_inputs(x_q, x_k, x_v, mask, Wq, bq, Wk, bk, Wv, bv, Wo, bo, gamma, beta):
    import ml_dtypes

    f = np.float32
    bf = ml_dtypes.bfloat16
    maskA = np.zeros((KB, QT), f)
    maskB = np.zeros((KB, QT), f)
    for i in range(KB):
        maskA[i, i:] = 1.0
        if i + 128 < QT:
            maskB[i, i + 128:] = 1.0
    mo = np.concatenate([maskA, maskB, maskA, maskB], axis=1).astype(bf)

    def chunk_xt(x_b):
        # x_b: [S, D] -> x^T [D, S] -> [r=4][p=128][cc=8][512]
        xt = np.ascontiguousarray(x_b.T.astype(bf))
        return np.ascontiguousarray(
            xt.reshape(8, 128, 4, 512).transpose(2, 1, 0, 3)
        )

    def perm_w(Wslice, ncols):
        # [ncols, D] weight slice -> W^T [D, ncols] -> [p=128][cc=8][ncols]
        wt = np.ascontiguousarray(Wslice.T.astype(bf))
        return np.ascontiguousarray(wt.reshape(8, 128, ncols).transpose(1, 0, 2))

    in_maps = []
    for c in range(NC):
        b, g = c // 4, c % 4
        dv = slice(DVC * g, DVC * (g + 1))
        smallc = np.zeros((128, 288), f)
        smallc[:, 0:2] = bq[dv].astype(f).reshape(2, 128).T
        smallc[:, 2:4] = bk[dv].astype(f).reshape(2, 128).T
        smallc[:, 4] = EPS
        smallc[:, 8 + 4 * b : 12 + 4 * b] = 1.0  # batch-select columns 8..15
        bvz = np.zeros((HPC, ND + 1), f)
        bvz[:, 0:ND] = bv[dv].astype(f).reshape(HPC, ND)
        bvz[:, ND] = 1.0
        smallc[:, 16 : 16 + HPC * (ND + 1)] = np.broadcast_to(
            bvz.reshape(-1), (128, HPC * (ND + 1))
        )
        res = (
            x_q[b, 512 * g : 512 * (g + 1), :].astype(f) + bo.astype(f)[None, :]
        )  # [512, D]
        res = np.ascontiguousarray(res.reshape(4, 128, D).transpose(1, 0, 2))
        in_maps.append(
            {
                "xtq": chunk_xt(x_q[b]),
                "xtk": chunk_xt(x_k[b]),
                "xtv": chunk_xt(x_v[b]),
                "wqT": perm_w(Wq[dv, :], DVC),
                "wkT": perm_w(Wk[dv, :], DVC),
                "wvT": perm_w(Wv[dv, :], DVC),
                "woT": perm_w(Wo, D),
                "smallc": smallc,
                "gam_bc": np.broadcast_to(gamma.astype(bf), (128, D)).copy(),
                "bet_bc": np.broadcast_to(beta.astype(bf), (128, D)).copy(),
                "resid": res,
                "mo": mo,
                "ones_r": np.ones((128, 64), f),
            }
        )
    return in_maps


def kernel(x_q, x_k, x_v, mask, Wq, bq, Wk, bk, Wv, bv, Wo, bo, gamma, beta):
    _install_ntff_shim()
    from concourse.bass_utils import run_bass_kernel_spmd

    x_q, x_k, x_v = np.asarray(x_q), np.asarray(x_k), np.asarray(x_v)
    mask = np.asarray(mask)
    # this kernel implements causal attention structurally; verify the mask
    causal = np.tril(np.ones((S, S), mask.dtype))
    assert np.array_equal(mask.reshape(S, S), causal), "kernel specialized for causal mask"

    if "nc" not in _cache:
        _cache["nc"] = _build()
    nc = _cache["nc"]

    in_maps = _prep_inputs(
        x_q, x_k, x_v, mask,
        np.asarray(Wq), np.asarray(bq), np.asarray(Wk), np.asarray(bk),
        np.asarray(Wv), np.asarray(bv), np.asarray(Wo), np.asarray(bo),
        np.asarray(gamma), np.asarray(beta),
    )
    res = run_bass_kernel_spmd(nc, in_maps, list(range(NC)))
    _cache["last_results"] = res

    out = np.empty((B, S, D), np.float32)
    for c in range(NC):
        b, g = c // 4, c % 4
        out[b, 512 * g : 512 * (g + 1), :] = res.results[c]["out"]
    return out
